# revision 1
# baseline (speedup 1.0000x reference)
"""Complementary gray-code structured-light decoder on 8 Trainium2 NeuronCores.

kernel(images: [24, 2048, 2448] f32) -> [2048, 2448, 2] f32

Sharding: H rows split across 8 cores (256 rows each), data-parallel, no
cross-core communication. Per core the 256x2448 slab is processed as 12
tiles of [128 rows x 408 cols].

Math (per pixel, both directions):
  s = (i1 + EPS*i2) - i3 ; c = (i0 + EPS*i1) - i2       (f32, == device einsum)
  thr = 0.125 * (i0+..+i7)  (sequential adds)
  mask = max(s_c^2+c_c^2, s_r^2+c_r^2) > T_EFF           (== 0.5*sqrt(q) > 0.05)
  b_i = gc_i > thr ; x_i = b_0 ^ .. ^ b_i                (gray decode)
  u = s*c / max(s^2, c^2)            (bounded atan argument, |u| <= 1;
                                      ACT Arctan's domain is [-pi/2, pi/2])
  sel = (s^2 > c^2)                  (atan(s/c) = sel ? sgn(u)*pi/2 - atan(u)
                                                      : atan(u))
  out = mask * ( -SCALE*atan(u) + 2*SCALE*sel*atan(u) - 4*sel*sgn(u)
                 + sum_{i=0..6} 2^(10-i) x_i + 8*x_7 + 8*x_7*sgn(c)
                 - 4*sgn(c) + 4 )
The last line is the algebraically collapsed unwrap: the phase wrap, the
atan2 quadrant adjustment and the k2/k1 region selection all reduce to
sign-of-c gated terms; no s-sign logic survives the cancellation.

Engines: PE does the threshold sum and the final linear combine (diagonal
matmuls, PSUM accumulation); DVE does the wide gray-vs-threshold compare
(stride-0 broadcast), the XOR cascade, and the mask multiply with
interleaved store; GPSIMD does f32 adds/squares/divide; ACT does the exact
affine scalings (EPS products, thr*0.125 PSUM evacuation) and Arctan.

All compare operands (s, c, thr, q) are bit-identical to the device
reference's own f32 arithmetic (validated on hardware), so mask and gray
decisions match the reference exactly; the only approximate decision is
(c<0) at atan2-output-rounding boundaries (~1 pixel per 5M, error <= 16 of
~2050 full-scale).
"""
import numpy as np

import concourse.bass as bass
import concourse.mybir as mybir
import concourse.tile as tile
from concourse.vector_clock import ScopedClock
from concourse.bass_utils import run_bass_kernel_spmd

# ---------------- constants ----------------
H, W = 2048, 2448
NFRAMES = 24
NCORES = 8
ROWS_PER_CORE = H // NCORES          # 256
F = 408                              # tile free width; W = 6*F
TILES_PER_CORE = (ROWS_PER_CORE // 128) * (W // F)   # 12

EPS = float(np.float32(8.742277657347586e-08))   # f32 sin(pi) == cos(pi/2) residue
T_EFF = 0.010000010952353477                     # (q > T_EFF) == (0.5*sqrt(q) > 0.05f)
SCALE = float(np.float32(16.0 / (2.0 * np.float64(np.pi))))
W_VALS = [1.0, -SCALE, 2.0 * SCALE]            # f32 diag weights
W_ID, W_NSC, W_2SC = 0, 1, 2
WB_VALS = [1024.0, 512.0, 256.0, 128.0, 64.0, 32.0, 16.0, 8.0, -4.0, 4.0]
WB_XI = [0, 1, 2, 3, 4, 5, 6]        # 2^(10-i) for x_i, i=0..6
WB_P8, WB_N4, WB_P4 = 7, 8, 9

f32 = mybir.dt.float32
bf16 = mybir.dt.bfloat16
OP = mybir.AluOpType
AF = mybir.ActivationFunctionType

_ctr = [0]


def _sanitize_waits(nc):
    """This walrus build rejects instructions carrying >1 sync wait. Move
    excess waits onto fresh same-engine NOPs inserted just before."""
    for f in nc.m.functions:
        for bb in f.blocks:
            il = bb.instructions
            i = 0
            while i < len(il):
                ins = il[i]
                si = getattr(ins, "sync_info", None)
                waits = list(si.on_wait) if si is not None and si.on_wait else []
                if len(waits) > 1:
                    si.on_wait = [waits[-1]]
                    ins.sync_info = si
                    for w in waits[:-1]:
                        _ctr[0] += 1
                        n = mybir.InstNoOp(name=f"waitsplit_{_ctr[0]}")
                        n.engine = ins.engine
                        n.sync_info = mybir.SyncInfo(on_wait=[w], on_update=[])
                        il.insert(i, n)
                        i += 1
                i += 1


class _SafeTileContext(tile.TileContext):
    """TileContext whose exit drain splits its sem waits across SP NOPs
    (the drain is emitted inside __exit__, before _sanitize_waits can run)."""

    def _drain_and_barrier(self, tick_clock, wait_clock):
        nop_inst = self.nc.sync.nop()
        wait_clock.add_sem_waits(
            nop_inst.ins, ScopedClock({None: tick_clock.global_clock})
        )
        si = nop_inst.ins.sync_info
        waits = list(si.on_wait) if si is not None else []
        if len(waits) > 1:
            si.on_wait = waits[:1]
            nop_inst.ins.sync_info = si
            for w in waits[1:]:
                n2 = self.nc.sync.nop()
                n2.ins.sync_info = mybir.SyncInfo(on_wait=[w], on_update=[])
        self.nc.sync.drain()

        self.nc.all_engine_barrier()
        assert self.sems is not None
        popped = self.nc._tile_sem_poison_stack.pop()
        assert popped is self._sem_poison
        self.nc.clear_and_free_semaphores(list(self.sems.allocated().values()))
        self.nc.all_engine_barrier()


def _build_program(sanitize=True):
    import contextlib

    nc = bass.Bass("TRN2", target_bir_lowering=False, debug=False)
    img = nc.dram_tensor("img", [NFRAMES, ROWS_PER_CORE, W], f32, kind="ExternalInput")
    wts = nc.dram_tensor("wts", [len(W_VALS), 128, 128], f32, kind="ExternalInput")
    wtsb = nc.dram_tensor("wtsb", [len(WB_VALS), 128, 128], bf16, kind="ExternalInput")
    out = nc.dram_tensor("out", [ROWS_PER_CORE, W, 2], f32, kind="ExternalOutput")

    with _SafeTileContext(nc) as tc, contextlib.ExitStack() as ctx:
        wpool = ctx.enter_context(tc.tile_pool(name="wpool", bufs=1))
        inp = ctx.enter_context(tc.tile_pool(name="inp", bufs=2))
        sb = ctx.enter_context(tc.tile_pool(name="sb", bufs=1))
        outp = ctx.enter_context(tc.tile_pool(name="outp", bufs=2))
        ps_s = ctx.enter_context(tc.tile_pool(name="ps_s", bufs=2, space="PSUM"))
        ps_t = ctx.enter_context(tc.tile_pool(name="ps_t", bufs=3, space="PSUM"))

        wt = wpool.tile([128, len(W_VALS) * 128], f32, tag="wt")
        for wi in range(len(W_VALS)):
            nc.sync.dma_start(out=wt[:, wi * 128:(wi + 1) * 128], in_=wts[wi, :, :])
        wtb = wpool.tile([128, len(WB_VALS) * 128], bf16, tag="wtb")
        for wi in range(len(WB_VALS)):
            nc.sync.dma_start(out=wtb[:, wi * 128:(wi + 1) * 128], in_=wtsb[wi, :, :])

        def wslot(i):
            return wt[:, i * 128:(i + 1) * 128]

        def wbslot(i):
            return wtb[:, i * 128:(i + 1) * 128]

        ones = wpool.tile([128, F], bf16, tag="ones")
        nc.vector.memset(ones[:, :], 1.0)

        for rb in range(ROWS_PER_CORE // 128):
            r0 = rb * 128
            for cb in range(W // F):
                c0 = cb * F
                # ---------------- load ----------------
                X = inp.tile([128, NFRAMES * F], f32, tag="X")
                nc.sync.dma_start(
                    out=X[:, :].rearrange("p (f x) -> p f x", f=NFRAMES),
                    in_=img[:, r0:r0 + 128, c0:c0 + F].rearrange("f p x -> p f x"),
                )

                def fr(i):
                    return X[:, i * F:(i + 1) * F]

                # ---------------- s, c (exact f32 einsum emulation) ----------------
                # eps pairs: [EPS*i1, EPS*i2] and [EPS*i5, EPS*i6] (frames adjacent)
                eps = sb.tile([128, 4 * F], f32, tag="eps")
                nc.scalar.activation(eps[:, 0:2 * F], X[:, F:3 * F], AF.Copy, bias=0.0, scale=EPS)
                nc.scalar.activation(eps[:, 2 * F:4 * F], X[:, 5 * F:7 * F], AF.Copy, bias=0.0, scale=EPS)

                # cs layout: [c_col | s_col | c_row | s_row]
                tmp = sb.tile([128, 4 * F], f32, tag="tmp")
                nc.gpsimd.tensor_tensor(tmp[:, 0:2 * F], X[:, 0:2 * F], eps[:, 0:2 * F], OP.add)
                nc.gpsimd.tensor_tensor(tmp[:, 2 * F:4 * F], X[:, 4 * F:6 * F], eps[:, 2 * F:4 * F], OP.add)
                cs = sb.tile([128, 4 * F], f32, tag="cs", bufs=2)
                nc.gpsimd.tensor_tensor(cs[:, 0:2 * F], tmp[:, 0:2 * F], X[:, 2 * F:4 * F], OP.subtract)
                nc.gpsimd.tensor_tensor(cs[:, 2 * F:4 * F], tmp[:, 2 * F:4 * F], X[:, 6 * F:8 * F], OP.subtract)

                def c_view():   # [128, 2, F] strided view of c planes (col, row)
                    return cs[:, :].rearrange("p (g t y) -> p g t y", g=2, t=2)[:, :, 0, :]

                def s_view():
                    return cs[:, :].rearrange("p (g t y) -> p g t y", g=2, t=2)[:, :, 1, :]

                # ---------------- threshold (PE sum + ACT scale) ----------------
                sum8 = ps_s.tile([128, F], f32, tag="sum8")
                for i in range(8):
                    nc.tensor.matmul(sum8[:, :], wslot(W_ID), fr(i), start=(i == 0), stop=(i == 7))
                thr = sb.tile([128, F], f32, tag="thr", bufs=2)
                nc.scalar.activation(thr[:, :], sum8[:, :], AF.Copy, bias=0.0, scale=0.125)

                # ---------------- gray compares + XOR cascade (DVE) ----------------
                # b layout: bit-pair-major — block (2i+d) holds bit i of dir d, so
                # each bit's (col,row) pair is one contiguous 2F run (2x DVE mode).
                b_all = sb.tile([128, 16 * F], bf16, tag="b_all", bufs=2)
                nc.vector.tensor_tensor(
                    b_all[:, :].rearrange("p (f d x) -> p d f x", f=8, d=2),
                    X[:, 8 * F:24 * F].rearrange("p (d f x) -> p d f x", d=2, f=8),
                    thr[:, :].rearrange("p (o v x) -> p o v x", o=1, v=1).broadcast_to([128, 2, 8, F]),
                    OP.is_gt,
                )

                x_all = sb.tile([128, 14 * F], bf16, tag="x_all", bufs=2)

                def xpair(i):  # contiguous [128, 2F] pair of cumulative bit i
                    if i == 0:
                        return b_all[:, 0:2 * F]
                    return x_all[:, (i - 1) * 2 * F:i * 2 * F]

                for i in range(1, 8):
                    nc.vector.tensor_tensor(
                        xpair(i), xpair(i - 1), b_all[:, 2 * i * F:(2 * i + 2) * F],
                        OP.logical_xor,
                    )

                def x_dir(i, d):
                    base = 0 if i == 0 else None
                    if i == 0:
                        return b_all[:, d * F:(d + 1) * F]
                    return x_all[:, ((i - 1) * 2 + d) * F:((i - 1) * 2 + d + 1) * F]

                # ---------------- squares, magnitude mask ----------------
                sq_s = sb.tile([128, 2 * F], f32, tag="sq_s")
                sq_c = sb.tile([128, 2 * F], f32, tag="sq_c")
                nc.scalar.activation(
                    sq_s[:, :].rearrange("p (d x) -> p d x", d=2),
                    s_view(), AF.Square, bias=0.0, scale=1.0,
                )
                nc.scalar.activation(
                    sq_c[:, :].rearrange("p (d x) -> p d x", d=2),
                    c_view(), AF.Square, bias=0.0, scale=1.0,
                )

                def sq_c_view():
                    return sq_c[:, :]

                def sq_s_view():
                    return sq_s[:, :]

                q_all = sb.tile([128, 2 * F], f32, tag="q_all")
                nc.gpsimd.tensor_tensor(q_all[:, :], sq_s[:, :], sq_c[:, :], OP.add)
                qm = sb.tile([128, F], f32, tag="qm")
                nc.vector.tensor_tensor(qm[:, :], q_all[:, 0:F], q_all[:, F:2 * F], OP.max)
                mask = sb.tile([128, F], f32, tag="mask")
                nc.vector.tensor_single_scalar(mask[:, :], qm[:, :], T_EFF, OP.is_gt)

                # ---------------- bounded atan path ----------------
                ma2 = sb.tile([128, 2 * F], f32, tag="ma2")
                nc.vector.tensor_tensor(ma2[:, :], sq_s[:, :], sq_c[:, :], OP.max)
                rcm = sb.tile([128, 2 * F], f32, tag="rcm")
                nc.vector.reciprocal(rcm[:, :], ma2[:, :])
                p_sc = sb.tile([128, 2 * F], f32, tag="p_sc")
                nc.gpsimd.tensor_tensor(
                    p_sc[:, :].rearrange("p (d x) -> p d x", d=2),
                    s_view(), c_view(), OP.mult,
                )
                u_all = sb.tile([128, 2 * F], f32, tag="u_all")
                nc.gpsimd.tensor_tensor(u_all[:, :], p_sc[:, :], rcm[:, :], OP.mult)
                a_u = sb.tile([128, 2 * F], f32, tag="a_u", bufs=2)
                nc.scalar.activation(a_u[:, :], u_all[:, :], AF.Arctan, bias=0.0, scale=1.0)
                sgn_u = sb.tile([128, 2 * F], bf16, tag="sgn_u")
                nc.scalar.activation(sgn_u[:, :], u_all[:, :], AF.Sign, bias=0.0, scale=1.0)
                sgn_c = sb.tile([128, 2 * F], bf16, tag="sgn_c")
                nc.scalar.activation(
                    sgn_c[:, :].rearrange("p (d x) -> p d x", d=2),
                    c_view(), AF.Sign, bias=0.0, scale=1.0,
                )

                sel = sb.tile([128, 2 * F], bf16, tag="sel")
                nc.vector.tensor_tensor(sel[:, :], sq_s[:, :], sq_c[:, :], OP.is_gt)
                sela = sb.tile([128, 2 * F], f32, tag="sela")
                nc.gpsimd.tensor_tensor(sela[:, :], sel[:, :], a_u[:, :], OP.mult)
                selsgn = sb.tile([128, 2 * F], bf16, tag="selsgn")
                nc.vector.tensor_tensor(selsgn[:, :], sel[:, :], sgn_u[:, :], OP.mult)
                x7sgn = sb.tile([128, 2 * F], bf16, tag="x7sgn")
                nc.vector.tensor_tensor(x7sgn[:, :], xpair(7), sgn_c[:, :], OP.mult)

                # ---------------- PE: t4 linear combine ----------------
                # t4 = -SCALE*a_u + 2*SCALE*sel*a_u - 4*sel*sgn_u
                #      + sum_i 2^(10-i)*x_i + 8*x7 + 8*x7*sgn_c - 4*sgn_c + 4
                t4 = [
                    ps_t.tile([128, F], f32, tag=f"t4_{d}", name=f"t4_{d}")
                    for d in range(2)
                ]
                for d in range(2):
                    sl = slice(d * F, (d + 1) * F)
                    nc.tensor.matmul(t4[d][:, :], wslot(W_NSC), a_u[:, sl], start=True, stop=False)
                    nc.tensor.matmul(t4[d][:, :], wslot(W_2SC), sela[:, sl], start=False, stop=False)
                    nc.tensor.matmul(t4[d][:, :], wbslot(WB_N4), selsgn[:, sl], start=False, stop=False)
                    for i in range(7):
                        nc.tensor.matmul(t4[d][:, :], wbslot(WB_XI[i]), x_dir(i, d), start=False, stop=False)
                    nc.tensor.matmul(t4[d][:, :], wbslot(WB_P8), x_dir(7, d), start=False, stop=False)
                    nc.tensor.matmul(t4[d][:, :], wbslot(WB_P8), x7sgn[:, sl], start=False, stop=False)
                    nc.tensor.matmul(t4[d][:, :], wbslot(WB_N4), sgn_c[:, sl], start=False, stop=False)
                    nc.tensor.matmul(t4[d][:, :], wbslot(WB_P4), ones[:, :], start=False, stop=True)

                # ---------------- mask multiply + interleaved store ----------------
                o_t = outp.tile([128, F * 2], f32, tag="o_t")
                ov = o_t[:, :].rearrange("p (x two) -> p x two", two=2)
                nc.vector.tensor_tensor(ov[:, :, 0], t4[0][:, :], mask[:, :], OP.mult)
                nc.vector.tensor_tensor(ov[:, :, 1], t4[1][:, :], mask[:, :], OP.mult)
                nc.sync.dma_start(
                    out=out[r0:r0 + 128, c0:c0 + F, :].rearrange("p x two -> p (x two)"),
                    in_=o_t[:, :],
                )

    if sanitize:
        _sanitize_waits(nc)
    return nc


def _weights():
    I = np.eye(128, dtype=np.float32)
    return np.stack([np.float32(v) * I for v in W_VALS]).astype(np.float32)


def _weights_b():
    import ml_dtypes
    I = np.eye(128, dtype=np.float32)
    return np.stack([np.float32(v) * I for v in WB_VALS]).astype(ml_dtypes.bfloat16)


_CACHE = {}


def _in_maps(images):
    wts = _weights()
    wtsb = _weights_b()
    maps = []
    for core in range(NCORES):
        r0 = core * ROWS_PER_CORE
        maps.append({
            "img": np.ascontiguousarray(images[:, r0:r0 + ROWS_PER_CORE, :]),
            "wts": wts,
            "wtsb": wtsb,
        })
    return maps


def kernel(images: np.ndarray) -> np.ndarray:
    images = np.ascontiguousarray(np.asarray(images, dtype=np.float32))
    assert images.shape == (NFRAMES, H, W), images.shape
    if "nc" not in _CACHE:
        _CACHE["nc"] = _build_program()
    res = run_bass_kernel_spmd(_CACHE["nc"], _in_maps(images), core_ids=list(range(NCORES)))
    out = np.empty((H, W, 2), dtype=np.float32)
    for core in range(NCORES):
        r0 = core * ROWS_PER_CORE
        out[r0:r0 + ROWS_PER_CORE] = res.results[core]["out"]
    return out


def timed_run(images: np.ndarray):
    """Run once with NTFF tracing; returns max per-core exec_time_ns or None."""
    images = np.ascontiguousarray(np.asarray(images, dtype=np.float32))
    if "nc" not in _CACHE:
        _CACHE["nc"] = _build_program()
    try:
        res = run_bass_kernel_spmd(
            _CACHE["nc"], _in_maps(images), core_ids=list(range(NCORES)),
            trace=True, trace_cores=[0],
        )
        return res.exec_time_ns
    except Exception as exc:
        print(f"timed_run: trace failed ({exc})")
        return None


if __name__ == "__main__":
    rng = np.random.default_rng(0)
    imgs = rng.random((NFRAMES, H, W), dtype=np.float32)
    o = kernel(imgs)
    print("ran:", o.shape, o.dtype, float(np.abs(o).max()))



# revision 10
# speedup vs baseline: 1.6065x; 1.6065x over previous
"""Complementary gray-code structured-light decoder on 8 Trainium2 NeuronCores.

kernel(images: [24, 2048, 2448] f32) -> [2048, 2448, 2] f32

Sharding: H rows split across 8 cores (256 rows each), data-parallel, no
cross-core communication. Per core the 256x2448 slab is processed as 12
tiles of [128 rows x 408 cols].

Math (per pixel, both directions d in {col,row}):
  s = i1 - i3 ; c = i0 - i2            (f32; the reference's EPS*sin(pi_f32)
                                        residue terms are ~8.7e-8 and only
                                        matter for |s| or |c| < 1e-7 — ~1
                                        pixel in 40M, error <= 24 of ~2050)
  thr  = 0.125 * (((i0+i1)+i2)+...+i7) (PE PSUM sequential f32 adds — exact)
  mask = max(s_c^2+c_c^2, s_r^2+c_r^2) > T_EFF    (== 0.5*sqrt(q) > 0.05f)
  b_i  = gc_i > thr ; x_i = b_0 ^ .. ^ b_i        (gray decode, exact f32)
  t    = atan(s * rcp|c|)  with rcp = magic-number approximate reciprocal
         (one int sub; |rel err| <= 5.1e-2 -> atan err <= 0.026; the ACT
         Arctan table is accurate over the full f32 range, probed on HW;
         |s|<1 and rcp<=1.62e38 keep the product finite)
  cpos = (c >= 0)
  out  = mask * ( SCALE*t - SCALE*(2*t*cpos)          [= -SCALE*sgn(c)*t
                                                       = -SCALE*atan(s/c)]
                 + sum_{i=0..6} 2^(10-i) x_i + 16*x_7*cpos + 8*(1-cpos) )
This is the same collapsed unwrap as the validated baseline formula
(-SCALE*t' + 2SCALE*sel*t' - 4*sel*sgn_u + ... + 8x7 + 8x7*sgn_c - 4sgn_c
 + 4, with t' = atan of the max-normalized ratio), rewritten through
atan(s/c) = sgn(c)*atan(s/|c|) and sgn_c = 2*cpos-1; algebraically
identical for c != 0, and the ACT Arctan table clamps large args.

Engines: PE does the threshold sum (f32, exact) and the final linear
combine (diagonal matmuls, bf16, PSUM accumulation, weight-grouped to
halve LDWEIGHTS); DVE does the wide gray-vs-threshold compare, the XOR
cascade (bf16 2x mode), the magic reciprocal (int ops), and the final
mask multiply; ACT does threshold evacuation, squares, Arctan and the
strided PSUM->SBUF interleave; GPSIMD does the s/c subtractions and the
magnitude add.
"""
import numpy as np

import concourse.bass as bass
import concourse.mybir as mybir
import concourse.tile as tile
from concourse.vector_clock import ScopedClock
from concourse.bass_utils import run_bass_kernel_spmd

# ---------------- constants ----------------
H, W = 2048, 2448
NFRAMES = 24
NCORES = 8
ROWS_PER_CORE = H // NCORES          # 256
F = 408                              # tile free width; W = 6*F
TILES_PER_CORE = (ROWS_PER_CORE // 128) * (W // F)   # 12

T_EFF = 0.010000010952353477         # (q > T_EFF) == (0.5*sqrt(q) > 0.05f)
SCALE = float(np.float32(16.0 / (2.0 * np.float64(np.pi))))
MAGIC = 0x7EF311C7                   # reciprocal seed: bits(1/x) ~ MAGIC - bits(x)
ZCLAMP = 512.0                       # atan arg clamp; atan(512) = pi/2 - 2e-3

# bf16 diagonal weight slots for the t4 linear combine
WB_VALS = [SCALE, -SCALE, 1024.0, 512.0, 256.0, 128.0, 64.0, 32.0, 16.0, -8.0, 8.0]
WB_T, WB_TC = 0, 1
WB_X = [2, 3, 4, 5, 6, 7, 8]         # 2^(10-i) for x_i, i=0..6
WB_X7 = 8                            # 16 * x7*cpos reuses the 16 slot
WB_NCP, WB_ONE = 9, 10

f32 = mybir.dt.float32
bf16 = mybir.dt.bfloat16
i32 = mybir.dt.int32
OP = mybir.AluOpType
AF = mybir.ActivationFunctionType

_ctr = [0]


def _sanitize_waits(nc):
    """This walrus build rejects instructions carrying >1 sync wait. Move
    excess waits onto fresh same-engine NOPs inserted just before."""
    for f in nc.m.functions:
        for bb in f.blocks:
            il = bb.instructions
            i = 0
            while i < len(il):
                ins = il[i]
                si = getattr(ins, "sync_info", None)
                waits = list(si.on_wait) if si is not None and si.on_wait else []
                if len(waits) > 1:
                    si.on_wait = [waits[-1]]
                    ins.sync_info = si
                    for w in waits[:-1]:
                        _ctr[0] += 1
                        n = mybir.InstNoOp(name=f"waitsplit_{_ctr[0]}")
                        n.engine = ins.engine
                        n.sync_info = mybir.SyncInfo(on_wait=[w], on_update=[])
                        il.insert(i, n)
                        i += 1
                i += 1


class _SafeTileContext(tile.TileContext):
    """TileContext whose exit drain splits its sem waits across SP NOPs
    (the drain is emitted inside __exit__, before _sanitize_waits can run)."""

    def _drain_and_barrier(self, tick_clock, wait_clock):
        nop_inst = self.nc.sync.nop()
        wait_clock.add_sem_waits(
            nop_inst.ins, ScopedClock({None: tick_clock.global_clock})
        )
        si = nop_inst.ins.sync_info
        waits = list(si.on_wait) if si is not None else []
        if len(waits) > 1:
            si.on_wait = waits[:1]
            nop_inst.ins.sync_info = si
            for w in waits[1:]:
                n2 = self.nc.sync.nop()
                n2.ins.sync_info = mybir.SyncInfo(on_wait=[w], on_update=[])
        self.nc.sync.drain()

        self.nc.all_engine_barrier()
        assert self.sems is not None
        popped = self.nc._tile_sem_poison_stack.pop()
        assert popped is self._sem_poison
        self.nc.clear_and_free_semaphores(list(self.sems.allocated().values()))
        self.nc.all_engine_barrier()


def _build_program(sanitize=True):
    import contextlib

    nc = bass.Bass("TRN2", target_bir_lowering=False, debug=False)
    img = nc.dram_tensor("img", [NFRAMES, ROWS_PER_CORE, W], f32, kind="ExternalInput")
    wid = nc.dram_tensor("wid", [128, 128], f32, kind="ExternalInput")
    wtsb = nc.dram_tensor("wtsb", [len(WB_VALS), 128, 128], bf16, kind="ExternalInput")
    out = nc.dram_tensor("out", [ROWS_PER_CORE, W, 2], f32, kind="ExternalOutput")

    with _SafeTileContext(nc) as tc, contextlib.ExitStack() as ctx:
        wpool = ctx.enter_context(tc.tile_pool(name="wpool", bufs=1))
        inp = ctx.enter_context(tc.tile_pool(name="inp", bufs=2))
        sb2 = ctx.enter_context(tc.tile_pool(name="sb2", bufs=2))
        sb1 = ctx.enter_context(tc.tile_pool(name="sb1", bufs=1))
        outp = ctx.enter_context(tc.tile_pool(name="outp", bufs=2))
        ps_s = ctx.enter_context(tc.tile_pool(name="ps_s", bufs=2, space="PSUM"))
        ps_t = ctx.enter_context(tc.tile_pool(name="ps_t", bufs=2, space="PSUM"))

        wI = wpool.tile([128, 128], f32, tag="wI")
        nc.sync.dma_start(out=wI[:, :], in_=wid[:, :])
        wtb = wpool.tile([128, len(WB_VALS) * 128], bf16, tag="wtb")
        for wi in range(len(WB_VALS)):
            nc.sync.dma_start(out=wtb[:, wi * 128:(wi + 1) * 128], in_=wtsb[wi, :, :])

        def wb(i):
            return wtb[:, i * 128:(i + 1) * 128]

        ones = wpool.tile([128, F], bf16, tag="ones")
        nc.vector.memset(ones[:, :], 1.0)

        for rb in range(ROWS_PER_CORE // 128):
            r0 = rb * 128
            for cb in range(W // F):
                c0 = cb * F
                # ---------------- load ----------------
                X = inp.tile([128, NFRAMES * F], f32, tag="X")
                nc.sync.dma_start(
                    out=X[:, :].rearrange("p (f x) -> p f x", f=NFRAMES),
                    in_=img[:, r0:r0 + 128, c0:c0 + F].rearrange("f p x -> p f x"),
                )

                def fr(i):
                    return X[:, i * F:(i + 1) * F]

                # ---------------- c, s (GPSIMD): cs = [c_col|s_col|c_row|s_row]
                cs = sb2.tile([128, 4 * F], f32, tag="cs")
                nc.gpsimd.tensor_tensor(cs[:, 0:2 * F], X[:, 0:2 * F], X[:, 2 * F:4 * F], OP.subtract)
                nc.gpsimd.tensor_tensor(cs[:, 2 * F:4 * F], X[:, 4 * F:6 * F], X[:, 6 * F:8 * F], OP.subtract)

                def c_view():   # [128, 2, F] strided view of c planes (col, row)
                    return cs[:, :].rearrange("p (d t x) -> p d t x", d=2, t=2)[:, :, 0, :]

                def s_view():
                    return cs[:, :].rearrange("p (d t x) -> p d t x", d=2, t=2)[:, :, 1, :]

                # ---------------- threshold (PE sum + ACT scale) ----------------
                sum8 = ps_s.tile([128, F], f32, tag="sum8")
                for i in range(8):
                    nc.tensor.matmul(sum8[:, :], wI[:, :], fr(i), start=(i == 0), stop=(i == 7))
                thr = sb2.tile([128, F], f32, tag="thr")
                nc.scalar.activation(thr[:, :], sum8[:, :], AF.Copy, bias=0.0, scale=0.125)

                # ---------------- gray compares + XOR cascade (DVE) ----------------
                # b_all natural plane order: col bits 0..7, row bits 0..7.
                b_all = sb2.tile([128, 16 * F], bf16, tag="b_all")
                nc.vector.tensor_tensor(
                    b_all[:, :].rearrange("p (f x) -> p f x", f=16),
                    X[:, 8 * F:24 * F].rearrange("p (f x) -> p f x", f=16),
                    thr[:, :].rearrange("p (o x) -> p o x", o=1).broadcast_to([128, 16, F]),
                    OP.is_gt,
                )

                # x_all pair-major: block i holds [x_{i+1}^col | x_{i+1}^row]
                x_all = sb2.tile([128, 14 * F], bf16, tag="x_all")

                def bpair(i):  # [128, 2, F] view of (col bit i, row bit i)
                    return b_all[:, :].rearrange("p (d f x) -> p f d x", d=2, f=8)[:, i]

                def xpair(i):  # [128, 2, F]-compatible cumulative pair for bit i
                    if i == 0:
                        return bpair(0)
                    return x_all[:, (i - 1) * 2 * F:i * 2 * F].rearrange("p (d x) -> p d x", d=2)

                for i in range(1, 8):
                    nc.vector.tensor_tensor(xpair(i), xpair(i - 1), bpair(i), OP.logical_xor)

                def x_dir(i, d):  # [128, F] cumulative bit i for direction d
                    if i == 0:
                        return b_all[:, d * 8 * F:(d * 8 + 1) * F]
                    return x_all[:, ((i - 1) * 2 + d) * F:((i - 1) * 2 + d + 1) * F]

                # ---------------- magnitude mask ----------------
                sq_s = sb1.tile([128, 2 * F], f32, tag="sq_s")
                sq_c = sb1.tile([128, 2 * F], f32, tag="sq_c")
                nc.scalar.activation(
                    sq_s[:, :].rearrange("p (d x) -> p d x", d=2),
                    s_view(), AF.Square, bias=0.0, scale=1.0,
                )
                nc.scalar.activation(
                    sq_c[:, :].rearrange("p (d x) -> p d x", d=2),
                    c_view(), AF.Square, bias=0.0, scale=1.0,
                )
                q_all = sb1.tile([128, 2 * F], f32, tag="q_all")
                nc.gpsimd.tensor_tensor(q_all[:, :], sq_s[:, :], sq_c[:, :], OP.add)
                qm = sb1.tile([128, F], f32, tag="qm")
                nc.vector.tensor_tensor(qm[:, :], q_all[:, 0:F], q_all[:, F:2 * F], OP.max)
                mask = sb1.tile([128, F], f32, tag="mask")
                nc.vector.tensor_scalar(mask[:, :], qm[:, :], T_EFF, None, OP.is_gt)

                # ---------------- atan(s/|c|) via magic reciprocal ----------------
                absC = sb1.tile([128, 2 * F], f32, tag="absC")
                nc.vector.tensor_scalar(
                    absC[:, :].rearrange("p (d x) -> p d x", d=2).bitcast(i32),
                    c_view().bitcast(i32), 0x7FFFFFFF, None, OP.bitwise_and,
                )
                rA = sb1.tile([128, 2 * F], f32, tag="rA")
                z = sb1.tile([128, 2 * F], f32, tag="z")
                ins_m = nc.vector.tensor_scalar(
                    rA[:, :].bitcast(i32), absC[:, :].bitcast(i32),
                    MAGIC, None, OP.subtract,
                )
                ins_m.ins.reverse0 = True
                nc.vector.tensor_tensor(
                    z[:, :].rearrange("p (d x) -> p d x", d=2),
                    s_view(),
                    rA[:, :].rearrange("p (d x) -> p d x", d=2),
                    OP.mult,
                )
                t_abs = sb1.tile([128, 2 * F], bf16, tag="t_abs")
                nc.scalar.activation(t_abs[:, :], z[:, :], AF.Arctan, bias=0.0, scale=1.0)

                # ---------------- cpos gates ----------------
                cpos = sb1.tile([128, 2 * F], bf16, tag="cpos")
                nc.vector.tensor_scalar(
                    cpos[:, :].rearrange("p (d x) -> p d x", d=2),
                    c_view(), 0.0, None, OP.is_ge,
                )
                tc2 = sb1.tile([128, 2 * F], bf16, tag="tc2")
                nc.vector.scalar_tensor_tensor(
                    tc2[:, :], t_abs[:, :], 2.0, cpos[:, :], OP.mult, OP.mult,
                )
                x7c = sb1.tile([128, 2 * F], bf16, tag="x7c")
                nc.vector.tensor_tensor(x7c[:, :], x_all[:, 12 * F:14 * F], cpos[:, :], OP.mult)

                # ---------------- PE: t4 linear combine (bf16, weight-grouped)
                t4 = [
                    ps_t.tile([128, F], f32, tag=f"t4_{d}", name=f"t4_{d}")
                    for d in range(2)
                ]

                def sl(ap, d):
                    return ap[:, d * F:(d + 1) * F]

                # decode terms first (ready earliest), then analog terms
                for d in range(2):
                    nc.tensor.matmul(t4[d][:, :], wb(WB_X[0]), x_dir(0, d), start=True, stop=False)
                for i in range(1, 7):
                    for d in range(2):
                        nc.tensor.matmul(t4[d][:, :], wb(WB_X[i]), x_dir(i, d), start=False, stop=False)
                for d in range(2):
                    nc.tensor.matmul(t4[d][:, :], wb(WB_X7), sl(x7c, d), start=False, stop=False)
                for d in range(2):
                    nc.tensor.matmul(t4[d][:, :], wb(WB_NCP), sl(cpos, d), start=False, stop=False)
                for d in range(2):
                    nc.tensor.matmul(t4[d][:, :], wb(WB_ONE), ones[:, :], start=False, stop=False)
                for d in range(2):
                    nc.tensor.matmul(t4[d][:, :], wb(WB_T), sl(t_abs, d), start=False, stop=False)
                for d in range(2):
                    nc.tensor.matmul(t4[d][:, :], wb(WB_TC), sl(tc2, d), start=False, stop=True)

                # ---------------- evacuate + mask + interleaved store ----------------
                o_t = outp.tile([128, F * 2], f32, tag="o_t")
                ev = outp.tile([128, F * 2], f32, tag="ev")
                evv = ev[:, :].rearrange("p (x two) -> p x two", two=2)
                for d in range(2):
                    nc.scalar.activation(evv[:, :, d], t4[d][:, :], AF.Copy, bias=0.0, scale=1.0)
                nc.vector.tensor_tensor(
                    o_t[:, :].rearrange("p (x two) -> p x two", two=2),
                    evv[:, :, :],
                    mask[:, :].rearrange("p (x o) -> p x o", o=1).broadcast_to([128, F, 2]),
                    OP.mult,
                )
                nc.sync.dma_start(
                    out=out[r0:r0 + 128, c0:c0 + F, :].rearrange("p x two -> p (x two)"),
                    in_=o_t[:, :],
                )

    if sanitize:
        _sanitize_waits(nc)
    return nc


def _weights_id():
    return np.eye(128, dtype=np.float32)


def _weights_b():
    import ml_dtypes
    I = np.eye(128, dtype=np.float32)
    return np.stack([np.float32(v) * I for v in WB_VALS]).astype(ml_dtypes.bfloat16)


_CACHE = {}


def _in_maps(images):
    wid = _weights_id()
    wtsb = _weights_b()
    maps = []
    for core in range(NCORES):
        r0 = core * ROWS_PER_CORE
        maps.append({
            "img": np.ascontiguousarray(images[:, r0:r0 + ROWS_PER_CORE, :]),
            "wid": wid,
            "wtsb": wtsb,
        })
    return maps


def kernel(images: np.ndarray) -> np.ndarray:
    images = np.ascontiguousarray(np.asarray(images, dtype=np.float32))
    assert images.shape == (NFRAMES, H, W), images.shape
    if "nc" not in _CACHE:
        _CACHE["nc"] = _build_program()
    res = run_bass_kernel_spmd(_CACHE["nc"], _in_maps(images), core_ids=list(range(NCORES)))
    out = np.empty((H, W, 2), dtype=np.float32)
    for core in range(NCORES):
        r0 = core * ROWS_PER_CORE
        out[r0:r0 + ROWS_PER_CORE] = res.results[core]["out"]
    return out


def timed_run(images: np.ndarray):
    """Run once with NTFF tracing; returns max per-core exec_time_ns or None."""
    images = np.ascontiguousarray(np.asarray(images, dtype=np.float32))
    if "nc" not in _CACHE:
        _CACHE["nc"] = _build_program()
    try:
        res = run_bass_kernel_spmd(
            _CACHE["nc"], _in_maps(images), core_ids=list(range(NCORES)),
            trace=True, trace_cores=[0],
        )
        return res.exec_time_ns
    except Exception as exc:
        print(f"timed_run: trace failed ({exc})")
        return None


if __name__ == "__main__":
    rng = np.random.default_rng(0)
    imgs = rng.random((NFRAMES, H, W), dtype=np.float32)
    o = kernel(imgs)
    print("ran:", o.shape, o.dtype, float(np.abs(o).max()))
